# revision 4
# baseline (speedup 1.0000x reference)
"""Trainium2 Bass kernel for nn_AttentiveBPNet (grouped attention scoring).

Math (exact algebraic reduction of the reference):
  The reference projects x -> x@W_att -> [N,H,C], then dots each head with
  att[:, :C] / att[:, C:].  That collapses to two tiny projections:
      sk = x @ wk,  sv = x @ wv         (wk/wv: [C,H] folded from W_att,att)
  Gathers at node_idxes, pairwise leaky-relu scores, mean over S, softmax
  over an axis of size M=2 (== sigmoid of the difference).

Distribution (8 cores):
  - x is sharded row-wise; each core projects its 25000 rows into a
    [rows,16] score table (s = [sk|sv]).
  - AllGather the table so every core holds all 200704 (padded) rows.
  - Groups (G=8192) are sharded 1024/core; per-group node indices are
    remapped on the host into positions of the device table layout and
    gathered with indirect DMA; scores/softmax computed on DVE/ACT.
"""

import numpy as np

import concourse.bacc as bacc
import concourse.bass as bass
import concourse.tile as tile
from concourse import mybir, bass_utils

# ---- problem constants (hardcoded; kernel.py must be self-contained) ----
NCORES = 8
N, C, H, M, S, G = 200000, 64, 8, 2, 16, 8192
SLOPE = 0.2
RPC = N // NCORES        # 25000 rows per core
JT = (RPC + 255) // 256  # 98 m-pair tiles per core
HALF = JT * 128          # 12544 rows per half
RPAD = 2 * HALF          # 25088 padded rows per core
GPC = G // NCORES        # 1024 groups per core
GT = GPC // 128          # 8 group-tiles per core
SBW = JT * 32            # 3136 f32 per partition in the local table
CH = 14                  # m-pairs per PSUM chunk
NCH = JT // CH           # 7 chunks
F32 = mybir.dt.float32
I32 = mybir.dt.int32

_cache: dict = {}


def _build_nc():
    nc = bacc.Bacc(trn_type="TRN2", num_devices=NCORES)
    xp = nc.declare_dram_parameter("xp", [128, HALF], F32, isOutput=False)
    w2d = nc.declare_dram_parameter("w2d", [128, 32], F32, isOutput=False)
    ikv = nc.declare_dram_parameter("ikv", [GT, 128, 64], I32, isOutput=False)
    yout = nc.declare_dram_parameter("yout", [GT, 128, 32], F32, isOutput=True)
    ag_in = nc.dram_tensor("ag_in", [128, SBW], F32)
    ag_out = nc.dram_tensor(
        "ag_out", [128 * NCORES, SBW], F32, addr_space="Shared"
    )

    with tile.TileContext(nc) as tc:
        with (
            tc.tile_pool(name="const", bufs=1) as cpool,
            tc.tile_pool(name="xin", bufs=3) as xpool,
            tc.tile_pool(name="psum", bufs=4, space="PSUM") as ppool,
            tc.tile_pool(name="stab", bufs=1) as spool,
            tc.tile_pool(name="gath", bufs=3) as gpool,
            tc.tile_pool(name="score", bufs=3) as zpool,
        ):
            # ---- phase A: project x shard into the local score table ----
            w2s = cpool.tile([128, 32], F32)
            nc.sync.dma_start(w2s[:, :], w2d[:, :])
            s_sb = spool.tile([128, SBW], F32)
            for q in range(NCH):
                xt = xpool.tile([128, CH * 128], F32)
                nc.sync.dma_start(
                    xt[:, :], xp[:, q * CH * 128 : (q + 1) * CH * 128]
                )
                ps = ppool.tile([128, CH * 32], F32)
                for k in range(CH):
                    nc.tensor.matmul(
                        ps[:, k * 32 : (k + 1) * 32],
                        lhsT=xt[:, k * 128 : (k + 1) * 128],
                        rhs=w2s[:, :],
                        start=True,
                        stop=True,
                    )
                nc.vector.tensor_copy(
                    s_sb[:, q * CH * 32 : (q + 1) * CH * 32], ps[:, :]
                )
            nc.sync.dma_start(ag_in[:, :], s_sb[:, :])

            # ---- phase B: share the table ----
            nc.gpsimd.collective_compute(
                "AllGather",
                mybir.AluOpType.bypass,
                replica_groups=[list(range(NCORES))],
                ins=[ag_in[:, :]],
                outs=[ag_out[:, :]],
            )
            s_rows = ag_out[:, :].rearrange("p (r c) -> (p r) c", c=16)

            # ---- phase C: gather + scores + softmax per 128-group tile ----
            for t in range(GT):
                ikv_sb = gpool.tile([128, 64], I32, tag="ikv")
                nc.sync.dma_start(ikv_sb[:, :], ikv[t, :, :])
                # HW indirect DMA consumes ONE offset per partition per
                # instruction (multi-index offset APs only work in the
                # simulator), so issue one [128,16] gather per k-slot.
                skg = gpool.tile([128, 512], F32, tag="skg")
                for k in range(M * S):
                    nc.gpsimd.indirect_dma_start(
                        out=skg[:, k * 16 : (k + 1) * 16],
                        out_offset=None,
                        in_=s_rows,
                        in_offset=bass.IndirectOffsetOnAxis(
                            ap=ikv_sb[:, k : k + 1], axis=0
                        ),
                    )
                svg = gpool.tile([128, 512], F32, tag="svg")
                for k in range(M * S):
                    nc.gpsimd.indirect_dma_start(
                        out=svg[:, k * 16 : (k + 1) * 16],
                        out_offset=None,
                        in_=s_rows,
                        in_offset=bass.IndirectOffsetOnAxis(
                            ap=ikv_sb[:, 32 + k : 33 + k], axis=0
                        ),
                    )
                # z[p, a, b, t, h] = sk[idxk[p,a,t], h] + sv[idxv[p,b,t], h]
                # (ISA allows at most 3 free dims per AP -> split over a)
                zk4 = skg[:, :].rearrange(
                    "p (a t c) -> p a t c", a=M, t=S, c=16
                )[:, :, :, 0:H]
                zv4 = svg[:, :].rearrange(
                    "p (b t c) -> p b t c", b=M, t=S, c=16
                )[:, :, :, H:16]
                z = zpool.tile([128, M * M * S * H], F32, tag="z")
                zw4 = z[:, :].rearrange(
                    "p (a b t c) -> p a b t c", a=M, b=M, t=S, c=H
                )
                for a in range(M):
                    nc.vector.tensor_tensor(
                        out=zw4[:, a],
                        in0=zk4[:, a].unsqueeze(1).broadcast_to([128, M, S, H]),
                        in1=zv4,
                        op=mybir.AluOpType.add,
                    )
                # sum over t of z and |z|:
                #   sum_t lrelu(z) = 0.6*sum_z + 0.4*sum_abs  (slope 0.2)
                zr = z[:, :].rearrange(
                    "p (a b t c) -> p (a b) c t", a=M, b=M, t=S, c=H
                )
                s_abs = zpool.tile([128, M * M * H], F32, tag="sabs")
                nc.vector.tensor_reduce(
                    out=s_abs[:, :],
                    in_=zr,
                    axis=mybir.AxisListType.X,
                    op=mybir.AluOpType.add,
                    apply_absolute_value=True,
                )
                s_z = zpool.tile([128, M * M * H], F32, tag="sz")
                nc.vector.tensor_reduce(
                    out=s_z[:, :],
                    in_=zr,
                    axis=mybir.AxisListType.X,
                    op=mybir.AluOpType.add,
                )
                # t2 = 1.5*sum_z + sum_abs ;  avg = 0.025 * t2
                t2 = zpool.tile([128, M * M * H], F32, tag="t2")
                nc.vector.tensor_scalar(
                    out=t2[:, :],
                    in0=s_z[:, :],
                    scalar1=1.5,
                    scalar2=None,
                    op0=mybir.AluOpType.mult,
                )
                nc.vector.tensor_tensor(
                    out=t2[:, :],
                    in0=t2[:, :],
                    in1=s_abs[:, :],
                    op=mybir.AluOpType.add,
                )
                # softmax over b (2 elems): p0 = sigmoid(0.025*(t2_b0-t2_b1))
                t2v = t2[:, :].rearrange("p (a b c) -> p a b c", a=M, b=M, c=H)
                d = zpool.tile([128, M * H], F32, tag="d")
                dv = d[:, :].rearrange("p (a c) -> p a c", a=M, c=H)
                nc.vector.tensor_tensor(
                    out=dv,
                    in0=t2v[:, :, 0, :],
                    in1=t2v[:, :, 1, :],
                    op=mybir.AluOpType.subtract,
                )
                out_t = zpool.tile([128, M * M * H], F32, tag="out")
                ov = out_t[:, :].rearrange(
                    "p (a b c) -> p a b c", a=M, b=M, c=H
                )
                nc.scalar.activation(
                    out=ov[:, :, 0, :],
                    in_=dv,
                    func=mybir.ActivationFunctionType.Sigmoid,
                    scale=SLOPE * 2.0 / ((M * S) // 2),
                )
                nc.vector.tensor_scalar(
                    out=ov[:, :, 1, :],
                    in0=ov[:, :, 0, :],
                    scalar1=-1.0,
                    scalar2=1.0,
                    op0=mybir.AluOpType.mult,
                    op1=mybir.AluOpType.add,
                )
                nc.sync.dma_start(yout[t, :, :], out_t[:, :])
    nc.finalize()
    return nc


def _fold_w2(W_att, att):
    Wr = W_att.reshape(C, H, C)
    wk = np.einsum("dhc,hc->dh", Wr, att[:, :C])
    wv = np.einsum("dhc,hc->dh", Wr, att[:, C:])
    return np.concatenate([wk, wv], axis=1).astype(np.float32)  # [C, 2H]


def _table_pos(n):
    """Map a global x-row index to its row in the device score table."""
    c, r = np.divmod(n, RPC)
    half, rr = np.divmod(r, HALF)
    j, m = np.divmod(rr, 128)
    return (c * RPAD + m * (2 * JT) + j * 2 + half).astype(np.int32)


def prepare_inputs(x, node_idxes, W_att, att):
    x = np.ascontiguousarray(np.asarray(x, dtype=np.float32))
    W_att = np.asarray(W_att, dtype=np.float32)
    att = np.asarray(att, dtype=np.float32)
    ni = np.asarray(node_idxes)

    W2 = _fold_w2(W_att, att)
    w2d = np.zeros((128, 32), np.float32)
    w2d[:C, :16] = W2
    w2d[C:, 16:] = W2

    xs = np.zeros((NCORES, RPAD, C), np.float32)
    xs[:, :RPC] = x.reshape(NCORES, RPC, C)
    xp = np.ascontiguousarray(
        xs.reshape(NCORES, 2, HALF, C).transpose(0, 1, 3, 2).reshape(
            NCORES, 128, HALF
        )
    )

    tp = _table_pos(ni)          # [G, M, 2, S] int32
    ik = tp[:, :, 1, :]          # key list, index a  -> sk
    iv = tp[:, :, 0, :]          # value list, index b -> sv
    ikv = np.empty((NCORES, GT, 128, 64), np.int32)
    ikv[..., 0:32] = ik.reshape(NCORES, GT, 128, M * S)
    ikv[..., 32:64] = iv.reshape(NCORES, GT, 128, M * S)

    in_maps = [
        {"xp": xp[c], "w2d": w2d, "ikv": ikv[c]} for c in range(NCORES)
    ]
    return in_maps


def kernel(x, edge_index, node_idxes, W_att, att, **_unused):
    in_maps = prepare_inputs(x, node_idxes, W_att, att)
    if "nc" not in _cache:
        _cache["nc"] = _build_nc()
    nc = _cache["nc"]
    import os

    trace = bool(int(os.environ.get("KERNEL_TRACE", "0")))
    res = bass_utils.run_bass_kernel_spmd(
        nc, in_maps, core_ids=list(range(NCORES)), trace=trace
    )
    _cache["last_result"] = res
    out = np.concatenate(
        [res.results[c]["yout"].reshape(GPC, M, M, H) for c in range(NCORES)],
        axis=0,
    )
    return out


# revision 5
# speedup vs baseline: 1.0082x; 1.0082x over previous
"""Trainium2 Bass kernel for nn_AttentiveBPNet (grouped attention scoring).

Math (exact algebraic reduction of the reference):
  The reference projects x -> x@W_att -> [N,H,C], then dots each head with
  att[:, :C] / att[:, C:].  That collapses to two tiny projections:
      sk = x @ wk,  sv = x @ wv         (wk/wv: [C,H] folded from W_att,att)
  Gathers at node_idxes, pairwise leaky-relu scores, mean over S, softmax
  over an axis of size M=2 (== sigmoid of the difference).

Distribution (8 cores):
  - x is sharded row-wise; each core projects its 25000 rows into a
    [rows,16] score table (s = [sk|sv]).
  - AllGather the table so every core holds all 200704 (padded) rows.
  - Groups (G=8192) are sharded 1024/core; per-group node indices are
    remapped on the host into positions of the device table layout and
    gathered with indirect DMA; scores/softmax computed on DVE/ACT.
"""

import numpy as np

import concourse.bacc as bacc
import concourse.bass as bass
import concourse.tile as tile
from concourse import mybir, bass_utils

# ---- problem constants (hardcoded; kernel.py must be self-contained) ----
NCORES = 8
N, C, H, M, S, G = 200000, 64, 8, 2, 16, 8192
SLOPE = 0.2
RPC = N // NCORES        # 25000 rows per core
JT = (RPC + 255) // 256  # 98 m-pair tiles per core
HALF = JT * 128          # 12544 rows per half
RPAD = 2 * HALF          # 25088 padded rows per core
GPC = G // NCORES        # 1024 groups per core
GT = GPC // 128          # 8 group-tiles per core
SBW = JT * 32            # 3136 f32 per partition in the local table
CH = 14                  # m-pairs per PSUM chunk
NCH = JT // CH           # 7 chunks
F32 = mybir.dt.float32
I32 = mybir.dt.int32

_cache: dict = {}


def _build_nc():
    nc = bacc.Bacc(trn_type="TRN2", num_devices=NCORES)
    xp = nc.declare_dram_parameter("xp", [128, HALF], F32, isOutput=False)
    w2d = nc.declare_dram_parameter("w2d", [128, 32], F32, isOutput=False)
    ikv = nc.declare_dram_parameter("ikv", [GT, 128, 64], I32, isOutput=False)
    yout = nc.declare_dram_parameter("yout", [GT, 128, 32], F32, isOutput=True)
    ag_in = nc.dram_tensor("ag_in", [128, SBW], F32)
    ag_out = nc.dram_tensor(
        "ag_out", [128 * NCORES, SBW], F32, addr_space="Shared"
    )

    with tile.TileContext(nc) as tc:
        with (
            tc.tile_pool(name="const", bufs=1) as cpool,
            tc.tile_pool(name="xin", bufs=3) as xpool,
            tc.tile_pool(name="psum", bufs=4, space="PSUM") as ppool,
            tc.tile_pool(name="stab", bufs=1) as spool,
            tc.tile_pool(name="gath", bufs=3) as gpool,
            tc.tile_pool(name="score", bufs=3) as zpool,
        ):
            # ---- phase A: project x shard into the local score table ----
            w2s = cpool.tile([128, 32], F32)
            nc.sync.dma_start(w2s[:, :], w2d[:, :])
            s_sb = spool.tile([128, SBW], F32)
            for q in range(NCH):
                xt = xpool.tile([128, CH * 128], F32)
                nc.sync.dma_start(
                    xt[:, :], xp[:, q * CH * 128 : (q + 1) * CH * 128]
                )
                ps = ppool.tile([128, CH * 32], F32)
                for k in range(CH):
                    nc.tensor.matmul(
                        ps[:, k * 32 : (k + 1) * 32],
                        lhsT=xt[:, k * 128 : (k + 1) * 128],
                        rhs=w2s[:, :],
                        start=True,
                        stop=True,
                    )
                nc.vector.tensor_copy(
                    s_sb[:, q * CH * 32 : (q + 1) * CH * 32], ps[:, :]
                )
            nc.sync.dma_start(ag_in[:, :], s_sb[:, :])

            # ---- phase B: share the table ----
            nc.gpsimd.collective_compute(
                "AllGather",
                mybir.AluOpType.bypass,
                replica_groups=[list(range(NCORES))],
                ins=[ag_in[:, :]],
                outs=[ag_out[:, :]],
            )
            s_rows = ag_out[:, :].rearrange("p (r c) -> (p r) c", c=16)

            # ---- phase C: gather + scores + softmax per 128-group tile ----
            for t in range(GT):
                ikv_sb = gpool.tile([128, 64], I32, tag="ikv")
                nc.sync.dma_start(ikv_sb[:, :], ikv[t, :, :])
                # HW indirect DMA consumes ONE offset per partition per
                # instruction (multi-index offset APs only work in the
                # simulator), so issue one [128,16] gather per k-slot.
                # Four independent destination tiles (one per a/b half)
                # decouple the DMA dependency chains for deeper pipelining.
                halves = []
                for half, (tag, base) in enumerate(
                    [("ska", 0), ("skb", S), ("sva", 32), ("svb", 32 + S)]
                ):
                    ht = gpool.tile([128, S * 16], F32, tag=tag)
                    for k in range(S):
                        nc.gpsimd.indirect_dma_start(
                            out=ht[:, k * 16 : (k + 1) * 16],
                            out_offset=None,
                            in_=s_rows,
                            in_offset=bass.IndirectOffsetOnAxis(
                                ap=ikv_sb[:, base + k : base + k + 1], axis=0
                            ),
                        )
                    halves.append(ht)
                ska, skb, sva, svb = halves
                # z[p, a, b, t, h] = sk[idxk[p,a,t], h] + sv[idxv[p,b,t], h]
                z = zpool.tile([128, M * M * S * H], F32, tag="z")
                for a in range(M):
                    skh = (ska, skb)[a]
                    kv = skh[:, :].rearrange(
                        "p (t c) -> p t c", t=S, c=16
                    )[:, :, 0:H]
                    for b in range(M):
                        svh = (sva, svb)[b]
                        vv = svh[:, :].rearrange(
                            "p (t c) -> p t c", t=S, c=16
                        )[:, :, H:16]
                        zslice = z[
                            :, (a * M + b) * S * H : (a * M + b + 1) * S * H
                        ].rearrange("p (t c) -> p t c", t=S, c=H)
                        nc.vector.tensor_tensor(
                            out=zslice, in0=kv, in1=vv, op=mybir.AluOpType.add
                        )
                # sum over t of z and |z|:
                #   sum_t lrelu(z) = 0.6*sum_z + 0.4*sum_abs  (slope 0.2)
                zr = z[:, :].rearrange(
                    "p (a b t c) -> p (a b) c t", a=M, b=M, t=S, c=H
                )
                s_abs = zpool.tile([128, M * M * H], F32, tag="sabs")
                nc.vector.tensor_reduce(
                    out=s_abs[:, :],
                    in_=zr,
                    axis=mybir.AxisListType.X,
                    op=mybir.AluOpType.add,
                    apply_absolute_value=True,
                )
                s_z = zpool.tile([128, M * M * H], F32, tag="sz")
                nc.vector.tensor_reduce(
                    out=s_z[:, :],
                    in_=zr,
                    axis=mybir.AxisListType.X,
                    op=mybir.AluOpType.add,
                )
                # t2 = 1.5*sum_z + sum_abs ;  avg = 0.025 * t2
                t2 = zpool.tile([128, M * M * H], F32, tag="t2")
                nc.vector.tensor_scalar(
                    out=t2[:, :],
                    in0=s_z[:, :],
                    scalar1=1.5,
                    scalar2=None,
                    op0=mybir.AluOpType.mult,
                )
                nc.vector.tensor_tensor(
                    out=t2[:, :],
                    in0=t2[:, :],
                    in1=s_abs[:, :],
                    op=mybir.AluOpType.add,
                )
                # softmax over b (2 elems): p0 = sigmoid(0.025*(t2_b0-t2_b1))
                t2v = t2[:, :].rearrange("p (a b c) -> p a b c", a=M, b=M, c=H)
                d = zpool.tile([128, M * H], F32, tag="d")
                dv = d[:, :].rearrange("p (a c) -> p a c", a=M, c=H)
                nc.vector.tensor_tensor(
                    out=dv,
                    in0=t2v[:, :, 0, :],
                    in1=t2v[:, :, 1, :],
                    op=mybir.AluOpType.subtract,
                )
                out_t = zpool.tile([128, M * M * H], F32, tag="out")
                ov = out_t[:, :].rearrange(
                    "p (a b c) -> p a b c", a=M, b=M, c=H
                )
                nc.scalar.activation(
                    out=ov[:, :, 0, :],
                    in_=dv,
                    func=mybir.ActivationFunctionType.Sigmoid,
                    scale=SLOPE * 2.0 / ((M * S) // 2),
                )
                nc.vector.tensor_scalar(
                    out=ov[:, :, 1, :],
                    in0=ov[:, :, 0, :],
                    scalar1=-1.0,
                    scalar2=1.0,
                    op0=mybir.AluOpType.mult,
                    op1=mybir.AluOpType.add,
                )
                nc.sync.dma_start(yout[t, :, :], out_t[:, :])
    nc.finalize()
    return nc


def _fold_w2(W_att, att):
    Wr = W_att.reshape(C, H, C)
    wk = np.einsum("dhc,hc->dh", Wr, att[:, :C])
    wv = np.einsum("dhc,hc->dh", Wr, att[:, C:])
    return np.concatenate([wk, wv], axis=1).astype(np.float32)  # [C, 2H]


def _table_pos(n):
    """Map a global x-row index to its row in the device score table."""
    c, r = np.divmod(n, RPC)
    half, rr = np.divmod(r, HALF)
    j, m = np.divmod(rr, 128)
    return (c * RPAD + m * (2 * JT) + j * 2 + half).astype(np.int32)


def prepare_inputs(x, node_idxes, W_att, att):
    x = np.ascontiguousarray(np.asarray(x, dtype=np.float32))
    W_att = np.asarray(W_att, dtype=np.float32)
    att = np.asarray(att, dtype=np.float32)
    ni = np.asarray(node_idxes)

    W2 = _fold_w2(W_att, att)
    w2d = np.zeros((128, 32), np.float32)
    w2d[:C, :16] = W2
    w2d[C:, 16:] = W2

    xs = np.zeros((NCORES, RPAD, C), np.float32)
    xs[:, :RPC] = x.reshape(NCORES, RPC, C)
    xp = np.ascontiguousarray(
        xs.reshape(NCORES, 2, HALF, C).transpose(0, 1, 3, 2).reshape(
            NCORES, 128, HALF
        )
    )

    tp = _table_pos(ni)          # [G, M, 2, S] int32
    ik = tp[:, :, 1, :]          # key list, index a  -> sk
    iv = tp[:, :, 0, :]          # value list, index b -> sv
    ikv = np.empty((NCORES, GT, 128, 64), np.int32)
    ikv[..., 0:32] = ik.reshape(NCORES, GT, 128, M * S)
    ikv[..., 32:64] = iv.reshape(NCORES, GT, 128, M * S)

    in_maps = [
        {"xp": xp[c], "w2d": w2d, "ikv": ikv[c]} for c in range(NCORES)
    ]
    return in_maps


def kernel(x, edge_index, node_idxes, W_att, att, **_unused):
    in_maps = prepare_inputs(x, node_idxes, W_att, att)
    if "nc" not in _cache:
        _cache["nc"] = _build_nc()
    nc = _cache["nc"]
    import os

    trace = bool(int(os.environ.get("KERNEL_TRACE", "0")))
    res = bass_utils.run_bass_kernel_spmd(
        nc, in_maps, core_ids=list(range(NCORES)), trace=trace
    )
    _cache["last_result"] = res
    out = np.concatenate(
        [res.results[c]["yout"].reshape(GPC, M, M, H) for c in range(NCORES)],
        axis=0,
    )
    return out
